# revision 8
# baseline (speedup 1.0000x reference)
"""Trainium2 Bass kernel for AttentionMLP (nn_AttentionMLP_72997264163220).

Reference computation:
  k/q/v = x @ W{k,q,v}.T + b      (D=3800 -> D)
  scores = q @ k.T / sqrt(D); attn = softmax(scores, -1)
  attended = attn @ v; h = attended.mean(seq)
  h = sigmoid(h @ W1.T + b1); h = sigmoid(h @ W2.T + b2); out = h @ W3.T + b3

Key algebraic simplification: the mean over the sequence commutes with
the attention matmul and the (linear) v projection,
  h = mean_i(attn) @ v = (abar @ x) @ Wv.T + bv,   abar = colmean_i(attn)
so v is never materialized: one [S]-vector per batch contracts x down to
a single [D]-vector before touching Wv. This removes ~1/3 of the matmul
work vs the naive dataflow.

Sharding: data-parallel over batch. 16 batches -> 8 cores x 2 batches
(512 tokens per core). All weights replicated, host pre-transposed /
tiled / cast. Big matmuls in bf16 (fp32 PSUM accumulate); softmax and
the MLP in fp32.

Device dataflow per core (SBUF partition dim always first; D padded to
3840 = 30*128 with a bias feature at d=3800):
  xT    [128, 30, 512] bf16  x^T (dp, kc, token); row d=3800 == 1
  x_tok [128, 4, 3840] bf16  x (token_p, token-tile, d); col d=3800 == 1
  per e-tile et in 30:  k_et/q_et [128,512] bf16  (q pre-scaled 1/sqrt(D))
     scores[2b+it] psum [128,256] += q_et_slice^T @ k_et_slice  over et
  softmax rows (fp32, on ACT/DVE) -> attn bf16 [128(i), 256(j)]
  abar[b] = colsum_i(attn)/S  via matmul with a const 1/S vector
  xa[b]   = abar[b] @ x       via x_tok   -> xaT [128, 30, 2] bf16 (xa[3800]=1)
  hT[et]  = Wv_tile^T @ xaT   (Wv has unit row at e=3800 -> hT[3800]=1)
  fp32 MLP; biases via the unit feature / unit rows, so no unaligned
  single-partition writes are ever needed.
"""

import sys
import types

import numpy as np

if "/opt/trn_rl_repo" not in sys.path:
    sys.path.insert(0, "/opt/trn_rl_repo")


# ---------------------------------------------------------------------------
# NTFF profile hook shim (antenv.axon_hooks is absent in this image). Needed
# only when profiling (trace=True); harmless otherwise.
# ---------------------------------------------------------------------------
def _install_ntff_hook():
    try:
        import antenv  # noqa: F401

        if "antenv.axon_hooks" in sys.modules:
            return
        hooks_mod = types.ModuleType("antenv.axon_hooks")
        hooks_mod._hook = None

        def set_axon_ntff_profile_hook(h):
            hooks_mod._hook = h

        def get_axon_ntff_profile_hook():
            return hooks_mod._hook

        hooks_mod.set_axon_ntff_profile_hook = set_axon_ntff_profile_hook
        hooks_mod.get_axon_ntff_profile_hook = get_axon_ntff_profile_hook
        sys.modules["antenv.axon_hooks"] = hooks_mod
        import antenv as _a

        _a.axon_hooks = hooks_mod
        from trn_agent_boot.trn_boot import _ntff_profile_via_ctypes

        set_axon_ntff_profile_hook(
            _ntff_profile_via_ctypes("/opt/axon/libaxon_pjrt.so")
        )
    except Exception:
        pass


_install_ntff_hook()


def _install_verbose_cc_hook():
    """Wrap the PJRT->python compile callback so real tracebacks surface
    instead of an opaque 'CallFunctionObjArgs' error."""
    try:
        import traceback

        from concourse import bass2jax

        bass2jax.install_neuronx_cc_hook()
        import libneuronxla

        if getattr(libneuronxla, "_ant_verbose_wrap", False):
            return
        orig = libneuronxla.neuronx_cc

        def wrapped(*a, **k):
            try:
                return orig(*a, **k)
            except BaseException:
                traceback.print_exc()
                sys.stderr.flush()
                raise

        libneuronxla.neuronx_cc = wrapped
        libneuronxla._ant_verbose_wrap = True
        bass2jax.install_neuronx_cc_hook = lambda: None
    except Exception:
        pass


import bass_rust
import ml_dtypes
import concourse.bass as bass
import concourse.tile as tile
from concourse import mybir
from concourse.bass_utils import run_bass_kernel_spmd
from concourse.vector_clock import ScopedClock

BF16 = ml_dtypes.bfloat16

N_CORES = 8
B = 16  # batches total
S = 256  # seq len
D = 3800  # feature dim
H = 512  # hidden
C = 10  # classes

BLOC = B // N_CORES  # batches per core = 2
T = BLOC * S  # tokens per core = 512
DP = 3840  # D padded (+1 bias feature, up to 30*128)
KC = DP // 128  # 30 contraction chunks
ET = DP // 128  # 30 e-tiles of 128
F32 = mybir.dt.float32
BF = mybir.dt.bfloat16


class SplitDrainTileContext(tile.TileContext):
    """This walrus build rejects >1 sync-wait on the tail Drain; split the
    global-clock waits across a chain of single-wait drain instructions."""

    MAXW = 1

    def _drain_and_barrier(self, tick_clock, wait_clock):
        nc = self.nc
        drain_inst = nc.sync.drain()
        wait_clock.add_sem_waits(
            drain_inst.ins, ScopedClock({None: tick_clock.global_clock})
        )
        si = drain_inst.ins.sync_info
        if si is not None and si.on_wait and len(si.on_wait) > self.MAXW:
            waits = list(si.on_wait)
            si.on_wait = waits[: self.MAXW]
            rest = waits[self.MAXW :]
            for i in range(0, len(rest), self.MAXW):
                extra = nc.sync.drain()
                extra.ins.sync_info = bass_rust.SyncInfo(
                    on_wait=rest[i : i + self.MAXW], on_update=[]
                )
        nc.all_engine_barrier()
        assert self.sems is not None
        popped = nc._tile_sem_poison_stack.pop()
        assert popped is self._sem_poison
        nc.clear_and_free_semaphores(list(self.sems.allocated().values()))
        nc.all_engine_barrier()


def _fix_excess_waits(nc, aux_sem, maxw=1):
    """Walrus in this image rejects instructions with more than ~1 sync
    wait. Compute-engine instructions: hoist extra waits onto same-engine
    no-ops inserted just before (sequencers execute in order). DMACopy:
    its waits live in the DGE queue descriptor, so an SP-side chain waits
    on all the original conditions, bumps `aux_sem`, and the descriptor
    waits on aux_sem alone."""
    aux_count = 0
    for f in nc.m.functions:
        for bb in f.blocks:
            insts = bb.instructions
            if not any(
                i.sync_info and i.sync_info.on_wait
                and len(i.sync_info.on_wait) > maxw
                for i in insts
            ):
                continue
            out = []
            for ins in insts:
                si = ins.sync_info
                nw = len(si.on_wait) if si and si.on_wait else 0
                if nw > maxw:
                    waits = list(si.on_wait)
                    if isinstance(ins, mybir.InstDMACopy):
                        for j, w in enumerate(waits):
                            nop = mybir.InstNoOp(name=f"{ins.name}-dw{j}")
                            nop.engine = mybir.EngineType.SP
                            nop.sync_info = bass_rust.SyncInfo(
                                on_wait=[w], on_update=[]
                            )
                            out.append(nop)
                        aux_count += 1
                        inc = mybir.InstNoOp(name=f"{ins.name}-dinc")
                        inc.engine = mybir.EngineType.SP
                        inc.sync_info = bass_rust.SyncInfo(
                            on_wait=[],
                            on_update=[
                                bass_rust.SyncUpdate(
                                    sync_type="semaphore",
                                    id=aux_sem.num,
                                    ant_name=aux_sem.name,
                                    update_mode="sem-add-imm",
                                    update_value=1,
                                    update_reg=None,
                                )
                            ],
                        )
                        out.append(inc)
                        si.on_wait = [
                            bass_rust.SyncWait(
                                sync_type="semaphore",
                                id=aux_sem.num,
                                ant_name=aux_sem.name,
                                wait_mode="sem-ge-imm",
                                wait_value=aux_count,
                                wait_reg=None,
                            )
                        ]
                    else:
                        keep = waits[-maxw:]
                        rest = waits[:-maxw]
                        for j, w in enumerate(rest):
                            nop = mybir.InstNoOp(name=f"{ins.name}-xw{j}")
                            nop.engine = ins.engine
                            nop.sync_info = bass_rust.SyncInfo(
                                on_wait=[w], on_update=[]
                            )
                            out.append(nop)
                        si.on_wait = keep
                out.append(ins)
            bb.instructions = out
    if aux_count:
        # reset aux sem at the very end so a re-executed NEFF starts clean
        f = nc.m.functions[0]
        bb = list(f.blocks)[-1]
        rst = mybir.InstNoOp(name="auxwait-reset")
        rst.engine = mybir.EngineType.SP
        rst.sync_info = bass_rust.SyncInfo(
            on_wait=[],
            on_update=[
                bass_rust.SyncUpdate(
                    sync_type="semaphore",
                    id=aux_sem.num,
                    ant_name=aux_sem.name,
                    update_mode="sem-sub-imm",
                    update_value=aux_count,
                    update_reg=None,
                )
            ],
        )
        il = bb.instructions
        il.append(rst)
        bb.instructions = il


def build_kernel() -> bass.Bass:
    nc = bass.Bass()

    x_d = nc.declare_dram_parameter("xT", [128, KC, T], BF, isOutput=False)
    xtok_d = nc.declare_dram_parameter("xtok", [128, 4, DP], BF, isOutput=False)
    wk_d = nc.declare_dram_parameter("wk", [ET, 128, KC, 128], BF, isOutput=False)
    wq_d = nc.declare_dram_parameter("wq", [ET, 128, KC, 128], BF, isOutput=False)
    wv_d = nc.declare_dram_parameter("wv", [ET, 128, KC, 128], BF, isOutput=False)
    w1_d = nc.declare_dram_parameter("w1", [128, KC, H], F32, isOutput=False)
    w2_d = nc.declare_dram_parameter("w2", [128, 5, H], F32, isOutput=False)
    w3_d = nc.declare_dram_parameter("w3", [128, 5, C], F32, isOutput=False)
    e0b_d = nc.declare_dram_parameter("e0b", [128, BLOC], F32, isOutput=False)
    out_d = nc.declare_dram_parameter("outT", [C, BLOC], F32, isOutput=True)

    aux_sem = nc.alloc_semaphore("auxwait")
    with SplitDrainTileContext(nc) as tc:
        with tc.tile_pool(name="persist", bufs=1) as persist:
            _emit(nc, tc, persist, x_d, xtok_d, wk_d, wq_d, wv_d, w1_d, w2_d,
                  w3_d, e0b_d, out_d)
    _fix_excess_waits(nc, aux_sem)
    return nc


def _emit(nc, tc, persist, x_d, xtok_d, wk_d, wq_d, wv_d, w1_d, w2_d, w3_d,
          e0b_d, out_d):
    # ------------------ persistent tiles ------------------
    xT = persist.tile([128, KC, T], BF)
    for kc in range(KC):
        nc.sync.dma_start(xT[:, kc, :], x_d[:, kc, :])
    x_tok = persist.tile([128, 4, DP], BF)
    for tt in range(4):
        nc.sync.dma_start(x_tok[:, tt, :], xtok_d[:, tt, :])
    ones_s = persist.tile([128, 1], BF)
    nc.vector.memset(ones_s[:], 1.0 / S)
    a_bar2 = persist.tile([128, 4, BLOC], BF)
    nc.vector.memset(a_bar2[:], 0.0)
    xaT = persist.tile([128, KC, BLOC], BF)
    hT = persist.tile([128, KC, BLOC], F32)

    # ------------- phase 1: k/q projections + score accumulation -------------
    with tc.tile_pool(name="psum_sc", bufs=1, space="PSUM") as psum_sc:
        ps = [
            psum_sc.tile([128, S], F32, name=f"scores{i}", tag=f"scores{i}")
            for i in range(4)  # index = 2*b + it
        ]
        with (
            tc.tile_pool(name="wkq", bufs=1) as wkq_pool,
            tc.tile_pool(name="kq_sb", bufs=1) as kq_sb,
            tc.tile_pool(name="psum_kq", bufs=1, space="PSUM") as psum_kq,
        ):
            for et in range(ET):
                wk_t = wkq_pool.tile([128, KC, 128], BF, tag="wk", bufs=2)
                nc.sync.dma_start(wk_t[:], wk_d[et])
                wq_t = wkq_pool.tile([128, KC, 128], BF, tag="wq", bufs=2)
                nc.sync.dma_start(wq_t[:], wq_d[et])

                pk = psum_kq.tile([128, T], F32, tag="pk", bufs=2)
                for kc in range(KC):
                    nc.tensor.matmul(
                        pk[:], wk_t[:, kc, :], xT[:, kc, :],
                        start=(kc == 0), stop=(kc == KC - 1),
                    )
                k_et = kq_sb.tile([128, T], BF, tag="k_et", bufs=2)
                nc.vector.tensor_copy(k_et[:], pk[:])

                pq = psum_kq.tile([128, T], F32, tag="pq", bufs=2)
                for kc in range(KC):
                    nc.tensor.matmul(
                        pq[:], wq_t[:, kc, :], xT[:, kc, :],
                        start=(kc == 0), stop=(kc == KC - 1),
                    )
                q_et = kq_sb.tile([128, T], BF, tag="q_et", bufs=2)
                nc.vector.tensor_copy(q_et[:], pq[:])

                for b in range(BLOC):
                    for it in range(2):
                        nc.tensor.matmul(
                            ps[2 * b + it][:],
                            q_et[:, b * S + it * 128 : b * S + (it + 1) * 128],
                            k_et[:, b * S : (b + 1) * S],
                            start=(et == 0), stop=(et == ET - 1),
                            skip_group_check=True,
                        )

        # ------------- phase 2: softmax + abar (column means) -------------
        with (
            tc.tile_pool(name="smx", bufs=1) as smx,
            tc.tile_pool(name="psum_ab", bufs=1, space="PSUM") as psum_ab,
        ):
            pab = [
                psum_ab.tile([128, 1], F32, name=f"pab{i}", tag=f"pab{i}")
                for i in range(4)  # index = 2*b + jc
            ]
            for b in range(BLOC):
                for it in range(2):
                    p = ps[2 * b + it]
                    mx = smx.tile([128, 1], F32, tag="mx", bufs=2)
                    nc.vector.reduce_max(
                        out=mx[:], in_=p[:], axis=mybir.AxisListType.X
                    )
                    negm = smx.tile([128, 1], F32, tag="negm", bufs=2)
                    nc.vector.tensor_scalar_mul(negm[:], mx[:], -1.0)
                    pexp = smx.tile([128, S], F32, tag="pexp", bufs=2)
                    sm = smx.tile([128, 1], F32, tag="sm", bufs=2)
                    nc.scalar.activation(
                        pexp[:], p[:], mybir.ActivationFunctionType.Exp,
                        bias=negm[:], accum_out=sm[:],
                    )
                    rin = smx.tile([128, 1], F32, tag="rin", bufs=2)
                    nc.vector.reciprocal(rin[:], sm[:])
                    attn_b = smx.tile([128, S], BF, tag="attn", bufs=2)
                    nc.vector.tensor_scalar_mul(attn_b[:], pexp[:], rin[:])
                    for jc in range(2):
                        nc.tensor.matmul(
                            pab[2 * b + jc][:],
                            attn_b[:, jc * 128 : (jc + 1) * 128],
                            ones_s[:],
                            start=(it == 0), stop=(it == 1),
                            skip_group_check=True,
                        )
            for b in range(BLOC):
                for jc in range(2):
                    nc.vector.tensor_copy(
                        a_bar2[:, 2 * b + jc, b : b + 1], pab[2 * b + jc][:]
                    )

    # ------------------ phase 3: xa = abar @ x ------------------
    with tc.tile_pool(name="psum_xa", bufs=1, space="PSUM") as psum_xa:
        for dt in range(KC):
            pxa = psum_xa.tile([128, BLOC], F32, tag="pxa", bufs=2)
            for tt in range(4):
                nc.tensor.matmul(
                    pxa[:],
                    x_tok[:, tt, dt * 128 : (dt + 1) * 128],
                    a_bar2[:, tt, :],
                    start=(tt == 0), stop=(tt == 3),
                )
            nc.vector.tensor_copy(xaT[:, dt, :], pxa[:])

    # ------------------ phase 4: hT = Wv^T-tiles @ xaT ------------------
    with (
        tc.tile_pool(name="wv", bufs=1) as wv_pool,
        tc.tile_pool(name="psum_h", bufs=1, space="PSUM") as psum_h,
    ):
        for et in range(ET):
            wv_t = wv_pool.tile([128, KC, 128], BF, tag="wv", bufs=4)
            nc.sync.dma_start(wv_t[:], wv_d[et])
            ph = psum_h.tile([128, BLOC], F32, tag="ph", bufs=2)
            for kc in range(KC):
                nc.tensor.matmul(
                    ph[:], wv_t[:, kc, :], xaT[:, kc, :],
                    start=(kc == 0), stop=(kc == KC - 1),
                )
            nc.vector.tensor_copy(hT[:, et, :], ph[:])

    # ------------------ phase 5: MLP (fp32) ------------------
    with (
        tc.tile_pool(name="mlpw", bufs=1) as mlpw,
        tc.tile_pool(name="mlph", bufs=1) as mlph,
        tc.tile_pool(name="psum_m", bufs=1, space="PSUM") as psum_m,
    ):
        w1_t = mlpw.tile([128, KC, H], F32)
        nc.sync.dma_start(w1_t[:], w1_d[:])
        w2_t = mlpw.tile([128, 5, H], F32)
        nc.sync.dma_start(w2_t[:], w2_d[:])
        w3_t = mlpw.tile([128, 5, C], F32)
        nc.sync.dma_start(w3_t[:], w3_d[:])

        h1T = mlph.tile([128, 5, BLOC], F32)
        nc.sync.dma_start(h1T[:, 4, :], e0b_d[:])
        for ot in range(4):
            pm = psum_m.tile([128, BLOC], F32, tag="pm1", bufs=2)
            for kc in range(KC):
                nc.tensor.matmul(
                    pm[:],
                    w1_t[:, kc, ot * 128 : (ot + 1) * 128],
                    hT[:, kc, :],
                    start=(kc == 0), stop=(kc == KC - 1),
                )
            nc.scalar.activation(
                h1T[:, ot, :], pm[:], mybir.ActivationFunctionType.Sigmoid
            )

        h2T = mlph.tile([128, 5, BLOC], F32)
        nc.sync.dma_start(h2T[:, 4, :], e0b_d[:])
        for ot in range(4):
            pm = psum_m.tile([128, BLOC], F32, tag="pm2", bufs=2)
            for oc in range(5):
                nc.tensor.matmul(
                    pm[:],
                    w2_t[:, oc, ot * 128 : (ot + 1) * 128],
                    h1T[:, oc, :],
                    start=(oc == 0), stop=(oc == 4),
                )
            nc.scalar.activation(
                h2T[:, ot, :], pm[:], mybir.ActivationFunctionType.Sigmoid
            )

        pm3 = psum_m.tile([C, BLOC], F32, tag="pm3")
        for oc in range(5):
            nc.tensor.matmul(
                pm3[:],
                w3_t[:, oc, :],
                h2T[:, oc, :],
                start=(oc == 0), stop=(oc == 4),
            )
        out_sb = mlph.tile([C, BLOC], F32)
        nc.vector.tensor_copy(out_sb[:], pm3[:])
        nc.sync.dma_start(out_d[:], out_sb[:])


# ---------------------------------------------------------------------------
# Host-side packing
# ---------------------------------------------------------------------------
def _pack_qkv_weight(W, bias, scale=1.0, unit_row=False):
    """W [D, D] (rows e out, cols d in), bias [D] -> [ET, 128, KC, 128] bf16
    with A[et, dp, kc, ep] = Wp[et*128+ep, kc*128+dp]; bias in column d=3800.
    unit_row: Wp[3800, 3800] = 1 so the padded out-row e=3800 reproduces the
    input's bias feature (used by Wv so hT keeps its unit feature)."""
    Wp = np.zeros((DP, DP), dtype=np.float32)
    Wp[:D, :D] = W if scale == 1.0 else W * scale
    Wp[:D, D] = bias if scale == 1.0 else bias * scale
    if unit_row:
        Wp[D, D] = 1.0
    A = Wp.reshape(ET, 128, KC, 128).transpose(0, 3, 2, 1)
    return np.ascontiguousarray(A, dtype=BF16)


def _pack_xT(xc):
    """xc [BLOC, S, D] -> [128, KC, T] bf16, bias row d=3800 = 1."""
    xt = np.zeros((DP, T), dtype=np.float32)
    xt[:D, :] = xc.reshape(T, D).T
    xt[D, :] = 1.0
    A = xt.reshape(KC, 128, T).transpose(1, 0, 2)
    return np.ascontiguousarray(A, dtype=BF16)


def _pack_xtok(xc):
    """xc [BLOC, S, D] -> [128, 4, DP] bf16 (token partition), col d=3800 = 1."""
    xp = np.zeros((T, DP), dtype=np.float32)
    xp[:, :D] = xc.reshape(T, D)
    xp[:, D] = 1.0
    A = xp.reshape(4, 128, DP).transpose(1, 0, 2)
    return np.ascontiguousarray(A, dtype=BF16)


def _pack_w1(W1, b1):
    """W1 [H, D] -> [128, KC, H] f32: A[dp, kc, o] = W1p[o, kc*128+dp];
    b1 in column d=3800 (hT[3800] == 1)."""
    W1p = np.zeros((H, DP), dtype=np.float32)
    W1p[:, :D] = W1
    W1p[:, D] = b1
    A = W1p.T.reshape(KC, 128, H).transpose(1, 0, 2)
    return np.ascontiguousarray(A, dtype=np.float32)


def _pack_w2(W2, b2):
    A = np.zeros((128, 5, H), dtype=np.float32)
    A[:, :4, :] = W2.T.reshape(4, 128, H).transpose(1, 0, 2)
    A[0, 4, :] = b2
    return np.ascontiguousarray(A)


def _pack_w3(W3, b3):
    A = np.zeros((128, 5, C), dtype=np.float32)
    A[:, :4, :] = W3.T.reshape(4, 128, C).transpose(1, 0, 2)
    A[0, 4, :] = b3
    return np.ascontiguousarray(A)


_NC_CACHE = {}


def _get_nc():
    if "nc" not in _NC_CACHE:
        _NC_CACHE["nc"] = build_kernel()
    return _NC_CACHE["nc"]


def kernel(x, Wk, bk, Wq, bq, Wv, bv, W1, b1, W2, b2, W3, b3, _trace=False):
    x = np.asarray(x, dtype=np.float32)
    scale = float(1.0 / np.sqrt(np.float32(D)))

    wk_p = _pack_qkv_weight(np.asarray(Wk, np.float32), np.asarray(bk, np.float32))
    wq_p = _pack_qkv_weight(
        np.asarray(Wq, np.float32), np.asarray(bq, np.float32), scale=scale
    )
    wv_p = _pack_qkv_weight(
        np.asarray(Wv, np.float32), np.asarray(bv, np.float32), unit_row=True
    )
    w1_p = _pack_w1(np.asarray(W1, np.float32), np.asarray(b1, np.float32))
    w2_p = _pack_w2(np.asarray(W2, np.float32), np.asarray(b2, np.float32))
    w3_p = _pack_w3(np.asarray(W3, np.float32), np.asarray(b3, np.float32))
    e0b = np.zeros((128, BLOC), dtype=np.float32)
    e0b[0, :] = 1.0

    in_maps = []
    for c in range(N_CORES):
        xc = x[c * BLOC : (c + 1) * BLOC]
        in_maps.append(
            {
                "xT": _pack_xT(xc),
                "xtok": _pack_xtok(xc),
                "wk": wk_p,
                "wq": wq_p,
                "wv": wv_p,
                "w1": w1_p,
                "w2": w2_p,
                "w3": w3_p,
                "e0b": e0b,
            }
        )

    nc = _get_nc()
    _install_verbose_cc_hook()
    res = run_bass_kernel_spmd(nc, in_maps, list(range(N_CORES)), trace=_trace)
    out = np.zeros((B, C), dtype=np.float32)
    for c in range(N_CORES):
        out[c * BLOC : (c + 1) * BLOC] = res.results[c]["outT"].T
    if _trace:
        return out, res
    return out


# revision 13
# speedup vs baseline: 1.2139x; 1.2139x over previous
"""Trainium2 Bass kernel for AttentionMLP (nn_AttentionMLP_72997264163220).

Reference computation:
  k/q/v = x @ W{k,q,v}.T + b      (D=3800 -> D)
  scores = q @ k.T / sqrt(D); attn = softmax(scores, -1)
  attended = attn @ v; h = attended.mean(seq)
  h = sigmoid(h @ W1.T + b1); h = sigmoid(h @ W2.T + b2); out = h @ W3.T + b3

Key algebraic simplification: the mean over the sequence commutes with
the attention matmul and the (linear) v projection,
  h = mean_i(attn) @ v = (abar @ x) @ Wv.T + bv,   abar = colmean_i(attn)
so v is never materialized: one [S]-vector per batch contracts x down to
a single [D]-vector before touching Wv. This removes ~1/3 of the matmul
work vs the naive dataflow.

Sharding: data-parallel over batch. 16 batches -> 8 cores x 2 batches
(512 tokens per core). All weights replicated, host pre-transposed /
tiled / cast. Big matmuls in bf16 (fp32 PSUM accumulate); softmax and
the MLP in fp32.

Device dataflow per core (SBUF partition dim always first; D padded to
3840 = 30*128 with a bias feature at d=3800):
  xT    [128, 30, 512] bf16  x^T (dp, kc, token); row d=3800 == 1
  x_tok [128, 4, 3840] bf16  x (token_p, token-tile, d); col d=3800 == 1
  per e-tile et in 30:  k_et/q_et [128,512] bf16  (q pre-scaled 1/sqrt(D))
     scores[2b+it] psum [128,256] += q_et_slice^T @ k_et_slice  over et
  softmax rows (fp32, on ACT/DVE) -> attn bf16 [128(i), 256(j)]
  abar[b] = colsum_i(attn)/S  via matmul with a const 1/S vector
  xa[b]   = abar[b] @ x       via x_tok   -> xaT [128, 30, 2] bf16 (xa[3800]=1)
  hT[et]  = Wv_tile^T @ xaT   (Wv has unit row at e=3800 -> hT[3800]=1)
  fp32 MLP; biases via the unit feature / unit rows, so no unaligned
  single-partition writes are ever needed.
"""

import sys
import types

import numpy as np

if "/opt/trn_rl_repo" not in sys.path:
    sys.path.insert(0, "/opt/trn_rl_repo")


# ---------------------------------------------------------------------------
# NTFF profile hook shim (antenv.axon_hooks is absent in this image). Needed
# only when profiling (trace=True); harmless otherwise.
# ---------------------------------------------------------------------------
def _install_ntff_hook():
    try:
        import antenv  # noqa: F401

        if "antenv.axon_hooks" in sys.modules:
            return
        hooks_mod = types.ModuleType("antenv.axon_hooks")
        hooks_mod._hook = None

        def set_axon_ntff_profile_hook(h):
            hooks_mod._hook = h

        def get_axon_ntff_profile_hook():
            return hooks_mod._hook

        hooks_mod.set_axon_ntff_profile_hook = set_axon_ntff_profile_hook
        hooks_mod.get_axon_ntff_profile_hook = get_axon_ntff_profile_hook
        sys.modules["antenv.axon_hooks"] = hooks_mod
        import antenv as _a

        _a.axon_hooks = hooks_mod
        from trn_agent_boot.trn_boot import _ntff_profile_via_ctypes

        set_axon_ntff_profile_hook(
            _ntff_profile_via_ctypes("/opt/axon/libaxon_pjrt.so")
        )
    except Exception:
        pass


_install_ntff_hook()


def _install_verbose_cc_hook():
    """Wrap the PJRT->python compile callback so real tracebacks surface
    instead of an opaque 'CallFunctionObjArgs' error."""
    try:
        import traceback

        from concourse import bass2jax

        bass2jax.install_neuronx_cc_hook()
        import libneuronxla

        if getattr(libneuronxla, "_ant_verbose_wrap", False):
            return
        orig = libneuronxla.neuronx_cc

        def wrapped(*a, **k):
            try:
                return orig(*a, **k)
            except BaseException:
                traceback.print_exc()
                sys.stderr.flush()
                raise

        libneuronxla.neuronx_cc = wrapped
        libneuronxla._ant_verbose_wrap = True
        bass2jax.install_neuronx_cc_hook = lambda: None
    except Exception:
        pass


import bass_rust
import ml_dtypes
import concourse.bass as bass
import concourse.tile as tile
from concourse import mybir
from concourse.bass_utils import run_bass_kernel_spmd
from concourse.vector_clock import ScopedClock

BF16 = ml_dtypes.bfloat16

N_CORES = 8
B = 16  # batches total
S = 256  # seq len
D = 3800  # feature dim
H = 512  # hidden
C = 10  # classes

BLOC = B // N_CORES  # batches per core = 2
T = BLOC * S  # tokens per core = 512
DP = 3840  # D padded (+1 bias feature, up to 30*128)
KC = DP // 128  # 30 contraction chunks
ET = DP // 128  # 30 e-tiles of 128
F32 = mybir.dt.float32
BF = mybir.dt.bfloat16


class SplitDrainTileContext(tile.TileContext):
    """This walrus build rejects >1 sync-wait on the tail Drain; split the
    global-clock waits across a chain of single-wait drain instructions."""

    MAXW = 1

    def _drain_and_barrier(self, tick_clock, wait_clock):
        nc = self.nc
        drain_inst = nc.sync.drain()
        wait_clock.add_sem_waits(
            drain_inst.ins, ScopedClock({None: tick_clock.global_clock})
        )
        si = drain_inst.ins.sync_info
        if si is not None and si.on_wait and len(si.on_wait) > self.MAXW:
            waits = list(si.on_wait)
            si.on_wait = waits[: self.MAXW]
            rest = waits[self.MAXW :]
            for i in range(0, len(rest), self.MAXW):
                extra = nc.sync.drain()
                extra.ins.sync_info = bass_rust.SyncInfo(
                    on_wait=rest[i : i + self.MAXW], on_update=[]
                )
        nc.all_engine_barrier()
        assert self.sems is not None
        popped = nc._tile_sem_poison_stack.pop()
        assert popped is self._sem_poison
        nc.clear_and_free_semaphores(list(self.sems.allocated().values()))
        nc.all_engine_barrier()


def _fix_excess_waits(nc, aux_sem, maxw=1):
    """Walrus in this image rejects instructions with more than ~1 sync
    wait. Compute-engine instructions: hoist extra waits onto same-engine
    no-ops inserted just before (sequencers execute in order). DMACopy:
    its waits live in the DGE queue descriptor, so an SP-side chain waits
    on all the original conditions, bumps `aux_sem`, and the descriptor
    waits on aux_sem alone."""
    aux_count = 0
    for f in nc.m.functions:
        for bb in f.blocks:
            insts = bb.instructions
            if not any(
                i.sync_info and i.sync_info.on_wait
                and len(i.sync_info.on_wait) > maxw
                for i in insts
            ):
                continue
            out = []
            for ins in insts:
                si = ins.sync_info
                nw = len(si.on_wait) if si and si.on_wait else 0
                if nw > maxw:
                    waits = list(si.on_wait)
                    if isinstance(ins, mybir.InstDMACopy):
                        for j, w in enumerate(waits):
                            nop = mybir.InstNoOp(name=f"{ins.name}-dw{j}")
                            nop.engine = mybir.EngineType.SP
                            nop.sync_info = bass_rust.SyncInfo(
                                on_wait=[w], on_update=[]
                            )
                            out.append(nop)
                        aux_count += 1
                        inc = mybir.InstNoOp(name=f"{ins.name}-dinc")
                        inc.engine = mybir.EngineType.SP
                        inc.sync_info = bass_rust.SyncInfo(
                            on_wait=[],
                            on_update=[
                                bass_rust.SyncUpdate(
                                    sync_type="semaphore",
                                    id=aux_sem.num,
                                    ant_name=aux_sem.name,
                                    update_mode="sem-add-imm",
                                    update_value=1,
                                    update_reg=None,
                                )
                            ],
                        )
                        out.append(inc)
                        si.on_wait = [
                            bass_rust.SyncWait(
                                sync_type="semaphore",
                                id=aux_sem.num,
                                ant_name=aux_sem.name,
                                wait_mode="sem-ge-imm",
                                wait_value=aux_count,
                                wait_reg=None,
                            )
                        ]
                    else:
                        keep = waits[-maxw:]
                        rest = waits[:-maxw]
                        for j, w in enumerate(rest):
                            nop = mybir.InstNoOp(name=f"{ins.name}-xw{j}")
                            nop.engine = ins.engine
                            nop.sync_info = bass_rust.SyncInfo(
                                on_wait=[w], on_update=[]
                            )
                            out.append(nop)
                        si.on_wait = keep
                out.append(ins)
            bb.instructions = out
    if aux_count:
        # reset aux sem at the very end so a re-executed NEFF starts clean
        f = nc.m.functions[0]
        bb = list(f.blocks)[-1]
        rst = mybir.InstNoOp(name="auxwait-reset")
        rst.engine = mybir.EngineType.SP
        rst.sync_info = bass_rust.SyncInfo(
            on_wait=[],
            on_update=[
                bass_rust.SyncUpdate(
                    sync_type="semaphore",
                    id=aux_sem.num,
                    ant_name=aux_sem.name,
                    update_mode="sem-sub-imm",
                    update_value=aux_count,
                    update_reg=None,
                )
            ],
        )
        il = bb.instructions
        il.append(rst)
        bb.instructions = il


def build_kernel() -> bass.Bass:
    nc = bass.Bass()

    x_d = nc.declare_dram_parameter("xT", [128, KC, T], BF, isOutput=False)
    xtok_d = nc.declare_dram_parameter("xtok", [128, 4, DP], BF, isOutput=False)
    wk_d = nc.declare_dram_parameter("wk", [ET, 128, KC, 128], BF, isOutput=False)
    wq_d = nc.declare_dram_parameter("wq", [ET, 128, KC, 128], BF, isOutput=False)
    wv_d = nc.declare_dram_parameter("wv", [ET, 128, KC, 128], BF, isOutput=False)
    w1_d = nc.declare_dram_parameter("w1", [128, KC, H], F32, isOutput=False)
    w2_d = nc.declare_dram_parameter("w2", [128, 5, H], F32, isOutput=False)
    w3_d = nc.declare_dram_parameter("w3", [128, 5, C], F32, isOutput=False)
    e0b_d = nc.declare_dram_parameter("e0b", [128, BLOC], F32, isOutput=False)
    out_d = nc.declare_dram_parameter("outT", [C, BLOC], F32, isOutput=True)

    aux_sem = nc.alloc_semaphore("auxwait")
    with SplitDrainTileContext(nc) as tc:
        with tc.tile_pool(name="persist", bufs=1) as persist:
            _emit(nc, tc, persist, x_d, xtok_d, wk_d, wq_d, wv_d, w1_d, w2_d,
                  w3_d, e0b_d, out_d)
    _fix_excess_waits(nc, aux_sem)
    return nc


def _emit(nc, tc, persist, x_d, xtok_d, wk_d, wq_d, wv_d, w1_d, w2_d, w3_d,
          e0b_d, out_d):
    # ------------------ persistent tiles ------------------
    # xT as one tile per contraction chunk so the first matmul only waits
    # for its own slice's DMA (Tile dependencies are whole-tile).
    xTc = [persist.tile([128, T], BF, name=f"xT{kc}", tag=f"xT{kc}")
           for kc in range(KC)]
    for kc in range(KC):
        nc.sync.dma_start(xTc[kc][:], x_d[:, kc, :])
    ones_s = persist.tile([128, 1], BF)
    nc.vector.memset(ones_s[:], 1.0 / S)
    a_bar2 = persist.tile([128, 4, BLOC], BF)
    nc.vector.memset(a_bar2[:], 0.0)
    x_tok = persist.tile([128, 4, DP], BF)
    xaT = persist.tile([128, KC, BLOC], BF)
    hT = persist.tile([128, KC, BLOC], F32)

    # MLP weights: tiles up-front, DMAs issued a few iterations into
    # phase 1 so they overlap compute instead of the critical startup
    mlpw = tc.alloc_tile_pool(name="mlpw", bufs=1)
    w1_t = mlpw.tile([128, KC, H], F32)
    w2_t = mlpw.tile([128, 5, H], F32)
    w3_t = mlpw.tile([128, 5, C], F32)

    # ------------- phase 1: k/q projections + score accumulation -------------
    with tc.tile_pool(name="psum_sc", bufs=1, space="PSUM") as psum_sc:
        ps = [
            psum_sc.tile([128, S], F32, name=f"scores{i}", tag=f"scores{i}")
            for i in range(4)  # index = 2*b + it
        ]
        with (
            tc.tile_pool(name="wkq", bufs=1) as wkq_pool,
            tc.tile_pool(name="kq_sb", bufs=1) as kq_sb,
            tc.tile_pool(name="psum_kq", bufs=1, space="PSUM") as psum_kq,
        ):
            for et in range(ET):
                wk_t = wkq_pool.tile([128, KC, 128], BF, tag="wk", bufs=2)
                nc.sync.dma_start(wk_t[:], wk_d[et])
                wq_t = wkq_pool.tile([128, KC, 128], BF, tag="wq", bufs=2)
                nc.sync.dma_start(wq_t[:], wq_d[et])
                if et == 3:
                    for tt in range(4):
                        nc.sync.dma_start(x_tok[:, tt, :], xtok_d[:, tt, :])
                    nc.sync.dma_start(w1_t[:], w1_d[:])
                    nc.sync.dma_start(w2_t[:], w2_d[:])
                    nc.sync.dma_start(w3_t[:], w3_d[:])

                pk = psum_kq.tile([128, T], F32, tag="pk", bufs=2)
                for kc in range(KC):
                    nc.tensor.matmul(
                        pk[:], wk_t[:, kc, :], xTc[kc][:],
                        start=(kc == 0), stop=(kc == KC - 1),
                    )
                k_et = kq_sb.tile([128, T], BF, tag="k_et", bufs=2)
                nc.vector.tensor_copy(k_et[:], pk[:])

                pq = psum_kq.tile([128, T], F32, tag="pq", bufs=2)
                for kc in range(KC):
                    nc.tensor.matmul(
                        pq[:], wq_t[:, kc, :], xTc[kc][:],
                        start=(kc == 0), stop=(kc == KC - 1),
                    )
                q_et = kq_sb.tile([128, T], BF, tag="q_et", bufs=2)
                nc.vector.tensor_copy(q_et[:], pq[:])

                for b in range(BLOC):
                    for it in range(2):
                        nc.tensor.matmul(
                            ps[2 * b + it][:],
                            q_et[:, b * S + it * 128 : b * S + (it + 1) * 128],
                            k_et[:, b * S : (b + 1) * S],
                            start=(et == 0), stop=(et == ET - 1),
                            skip_group_check=True,
                        )

        # ------------- phase 2: softmax + abar (column means) -------------
        with (
            tc.tile_pool(name="smx", bufs=1) as smx,
            tc.tile_pool(name="psum_ab", bufs=1, space="PSUM") as psum_ab,
        ):
            pab = [
                psum_ab.tile([128, 1], F32, name=f"pab{i}", tag=f"pab{i}")
                for i in range(4)  # index = 2*b + jc
            ]
            for b in range(BLOC):
                for it in range(2):
                    p = ps[2 * b + it]
                    mx = smx.tile([128, 1], F32, tag="mx", bufs=2)
                    nc.vector.reduce_max(
                        out=mx[:], in_=p[:], axis=mybir.AxisListType.X
                    )
                    negm = smx.tile([128, 1], F32, tag="negm", bufs=2)
                    nc.vector.tensor_scalar_mul(negm[:], mx[:], -1.0)
                    pexp = smx.tile([128, S], F32, tag="pexp", bufs=2)
                    sm = smx.tile([128, 1], F32, tag="sm", bufs=2)
                    nc.scalar.activation(
                        pexp[:], p[:], mybir.ActivationFunctionType.Exp,
                        bias=negm[:], accum_out=sm[:],
                    )
                    rin = smx.tile([128, 1], F32, tag="rin", bufs=2)
                    nc.vector.reciprocal(rin[:], sm[:])
                    attn_b = smx.tile([128, S], BF, tag="attn", bufs=2)
                    nc.vector.tensor_scalar_mul(attn_b[:], pexp[:], rin[:])
                    for jc in range(2):
                        nc.tensor.matmul(
                            pab[2 * b + jc][:],
                            attn_b[:, jc * 128 : (jc + 1) * 128],
                            ones_s[:],
                            start=(it == 0), stop=(it == 1),
                            skip_group_check=True,
                        )
            for b in range(BLOC):
                for jc in range(2):
                    nc.vector.tensor_copy(
                        a_bar2[:, 2 * b + jc, b : b + 1], pab[2 * b + jc][:]
                    )

    # ------------------ phase 3: xa = abar @ x ------------------
    with tc.tile_pool(name="psum_xa", bufs=1, space="PSUM") as psum_xa:
        for dt in range(KC):
            pxa = psum_xa.tile([128, BLOC], F32, tag="pxa", bufs=2)
            for tt in range(4):
                nc.tensor.matmul(
                    pxa[:],
                    x_tok[:, tt, dt * 128 : (dt + 1) * 128],
                    a_bar2[:, tt, :],
                    start=(tt == 0), stop=(tt == 3),
                )
            nc.vector.tensor_copy(xaT[:, dt, :], pxa[:])

    # ------------------ phase 4: hT = Wv^T-tiles @ xaT ------------------
    with (
        tc.tile_pool(name="wv", bufs=1) as wv_pool,
        tc.tile_pool(name="psum_h", bufs=1, space="PSUM") as psum_h,
    ):
        for et in range(ET):
            wv_t = wv_pool.tile([128, KC, 128], BF, tag="wv", bufs=6)
            nc.sync.dma_start(wv_t[:], wv_d[et])
            ph = psum_h.tile([128, BLOC], F32, tag="ph", bufs=2)
            for kc in range(KC):
                nc.tensor.matmul(
                    ph[:], wv_t[:, kc, :], xaT[:, kc, :],
                    start=(kc == 0), stop=(kc == KC - 1),
                )
            nc.vector.tensor_copy(hT[:, et, :], ph[:])

    # ------------------ phase 5: MLP (fp32) ------------------
    with (
        tc.tile_pool(name="mlph", bufs=1) as mlph,
        tc.tile_pool(name="psum_m", bufs=1, space="PSUM") as psum_m,
    ):
        h1T = mlph.tile([128, 5, BLOC], F32)
        nc.sync.dma_start(h1T[:, 4, :], e0b_d[:])
        for ot in range(4):
            pm = psum_m.tile([128, BLOC], F32, tag="pm1", bufs=2)
            for kc in range(KC):
                nc.tensor.matmul(
                    pm[:],
                    w1_t[:, kc, ot * 128 : (ot + 1) * 128],
                    hT[:, kc, :],
                    start=(kc == 0), stop=(kc == KC - 1),
                )
            nc.scalar.activation(
                h1T[:, ot, :], pm[:], mybir.ActivationFunctionType.Sigmoid
            )

        h2T = mlph.tile([128, 5, BLOC], F32)
        nc.sync.dma_start(h2T[:, 4, :], e0b_d[:])
        for ot in range(4):
            pm = psum_m.tile([128, BLOC], F32, tag="pm2", bufs=2)
            for oc in range(5):
                nc.tensor.matmul(
                    pm[:],
                    w2_t[:, oc, ot * 128 : (ot + 1) * 128],
                    h1T[:, oc, :],
                    start=(oc == 0), stop=(oc == 4),
                )
            nc.scalar.activation(
                h2T[:, ot, :], pm[:], mybir.ActivationFunctionType.Sigmoid
            )

        pm3 = psum_m.tile([C, BLOC], F32, tag="pm3")
        for oc in range(5):
            nc.tensor.matmul(
                pm3[:],
                w3_t[:, oc, :],
                h2T[:, oc, :],
                start=(oc == 0), stop=(oc == 4),
            )
        out_sb = mlph.tile([C, BLOC], F32)
        nc.vector.tensor_copy(out_sb[:], pm3[:])
        nc.sync.dma_start(out_d[:], out_sb[:])
    mlpw.release()


# ---------------------------------------------------------------------------
# Host-side packing
# ---------------------------------------------------------------------------
def _pack_qkv_weight(W, bias, scale=1.0, unit_row=False):
    """W [D, D] (rows e out, cols d in), bias [D] -> [ET, 128, KC, 128] bf16
    with A[et, dp, kc, ep] = Wp[et*128+ep, kc*128+dp]; bias in column d=3800.
    unit_row: Wp[3800, 3800] = 1 so the padded out-row e=3800 reproduces the
    input's bias feature (used by Wv so hT keeps its unit feature)."""
    Wp = np.zeros((DP, DP), dtype=np.float32)
    Wp[:D, :D] = W if scale == 1.0 else W * scale
    Wp[:D, D] = bias if scale == 1.0 else bias * scale
    if unit_row:
        Wp[D, D] = 1.0
    A = Wp.reshape(ET, 128, KC, 128).transpose(0, 3, 2, 1)
    return np.ascontiguousarray(A, dtype=BF16)


def _pack_xT(xc):
    """xc [BLOC, S, D] -> [128, KC, T] bf16, bias row d=3800 = 1."""
    xt = np.zeros((DP, T), dtype=np.float32)
    xt[:D, :] = xc.reshape(T, D).T
    xt[D, :] = 1.0
    A = xt.reshape(KC, 128, T).transpose(1, 0, 2)
    return np.ascontiguousarray(A, dtype=BF16)


def _pack_xtok(xc):
    """xc [BLOC, S, D] -> [128, 4, DP] bf16 (token partition), col d=3800 = 1."""
    xp = np.zeros((T, DP), dtype=np.float32)
    xp[:, :D] = xc.reshape(T, D)
    xp[:, D] = 1.0
    A = xp.reshape(4, 128, DP).transpose(1, 0, 2)
    return np.ascontiguousarray(A, dtype=BF16)


def _pack_w1(W1, b1):
    """W1 [H, D] -> [128, KC, H] f32: A[dp, kc, o] = W1p[o, kc*128+dp];
    b1 in column d=3800 (hT[3800] == 1)."""
    W1p = np.zeros((H, DP), dtype=np.float32)
    W1p[:, :D] = W1
    W1p[:, D] = b1
    A = W1p.T.reshape(KC, 128, H).transpose(1, 0, 2)
    return np.ascontiguousarray(A, dtype=np.float32)


def _pack_w2(W2, b2):
    A = np.zeros((128, 5, H), dtype=np.float32)
    A[:, :4, :] = W2.T.reshape(4, 128, H).transpose(1, 0, 2)
    A[0, 4, :] = b2
    return np.ascontiguousarray(A)


def _pack_w3(W3, b3):
    A = np.zeros((128, 5, C), dtype=np.float32)
    A[:, :4, :] = W3.T.reshape(4, 128, C).transpose(1, 0, 2)
    A[0, 4, :] = b3
    return np.ascontiguousarray(A)


_NC_CACHE = {}


def _get_nc():
    if "nc" not in _NC_CACHE:
        _NC_CACHE["nc"] = build_kernel()
    return _NC_CACHE["nc"]


def kernel(x, Wk, bk, Wq, bq, Wv, bv, W1, b1, W2, b2, W3, b3, _trace=False):
    x = np.asarray(x, dtype=np.float32)
    scale = float(1.0 / np.sqrt(np.float32(D)))

    wk_p = _pack_qkv_weight(np.asarray(Wk, np.float32), np.asarray(bk, np.float32))
    wq_p = _pack_qkv_weight(
        np.asarray(Wq, np.float32), np.asarray(bq, np.float32), scale=scale
    )
    wv_p = _pack_qkv_weight(
        np.asarray(Wv, np.float32), np.asarray(bv, np.float32), unit_row=True
    )
    w1_p = _pack_w1(np.asarray(W1, np.float32), np.asarray(b1, np.float32))
    w2_p = _pack_w2(np.asarray(W2, np.float32), np.asarray(b2, np.float32))
    w3_p = _pack_w3(np.asarray(W3, np.float32), np.asarray(b3, np.float32))
    e0b = np.zeros((128, BLOC), dtype=np.float32)
    e0b[0, :] = 1.0

    in_maps = []
    for c in range(N_CORES):
        xc = x[c * BLOC : (c + 1) * BLOC]
        in_maps.append(
            {
                "xT": _pack_xT(xc),
                "xtok": _pack_xtok(xc),
                "wk": wk_p,
                "wq": wq_p,
                "wv": wv_p,
                "w1": w1_p,
                "w2": w2_p,
                "w3": w3_p,
                "e0b": e0b,
            }
        )

    nc = _get_nc()
    _install_verbose_cc_hook()
    res = run_bass_kernel_spmd(nc, in_maps, list(range(N_CORES)), trace=_trace)
    out = np.zeros((B, C), dtype=np.float32)
    for c in range(N_CORES):
        out[c * BLOC : (c + 1) * BLOC] = res.results[c]["outT"].T
    if _trace:
        return out, res
    return out


# revision 20
# speedup vs baseline: 2.3800x; 1.9607x over previous
"""Trainium2 Bass kernel for AttentionMLP (nn_AttentionMLP_72997264163220).

Reference computation:
  k/q/v = x @ W{k,q,v}.T + b      (D=3800 -> D)
  scores = q @ k.T / sqrt(D); attn = softmax(scores, -1)
  attended = attn @ v; h = attended.mean(seq)
  h = sigmoid(h @ W1.T + b1); h = sigmoid(h @ W2.T + b2); out = h @ W3.T + b3

Key algebraic simplification: the mean over the sequence commutes with
the attention matmul and the (linear) v projection,
  h = mean_i(attn) @ v = (abar @ x) @ Wv.T + bv,   abar = colmean_i(attn)
so v is never materialized: one [S]-vector per batch contracts x down to
a single [D]-vector before touching Wv. This removes ~1/3 of the matmul
work vs the naive dataflow.

Sharding: data-parallel over batch. 16 batches -> 8 cores x 2 batches
(512 tokens per core). All weights replicated, host pre-transposed /
tiled / cast. Big matmuls in bf16 (fp32 PSUM accumulate); softmax and
the MLP in fp32.

Device dataflow per core (SBUF partition dim always first; D padded to
3840 = 30*128 with a bias feature at d=3800):
  xT    [128, 30, 512] bf16  x^T (dp, kc, token); row d=3800 == 1
  x_tok [128, 4, 3840] bf16  x (token_p, token-tile, d); col d=3800 == 1
  per e-tile et in 30:  k_et/q_et [128,512] bf16  (q pre-scaled 1/sqrt(D))
     scores[2b+it] psum [128,256] += q_et_slice^T @ k_et_slice  over et
  softmax rows (fp32, on ACT/DVE) -> attn bf16 [128(i), 256(j)]
  abar[b] = colsum_i(attn)/S  via matmul with a const 1/S vector
  xa[b]   = abar[b] @ x       via x_tok   -> xaT [128, 30, 2] bf16 (xa[3800]=1)
  hT[et]  = Wv_tile^T @ xaT   (Wv has unit row at e=3800 -> hT[3800]=1)
  fp32 MLP; biases via the unit feature / unit rows, so no unaligned
  single-partition writes are ever needed.
"""

import sys
import types

import numpy as np

if "/opt/trn_rl_repo" not in sys.path:
    sys.path.insert(0, "/opt/trn_rl_repo")


# ---------------------------------------------------------------------------
# NTFF profile hook shim (antenv.axon_hooks is absent in this image). Needed
# only when profiling (trace=True); harmless otherwise.
# ---------------------------------------------------------------------------
def _install_ntff_hook():
    try:
        import antenv  # noqa: F401

        if "antenv.axon_hooks" in sys.modules:
            return
        hooks_mod = types.ModuleType("antenv.axon_hooks")
        hooks_mod._hook = None

        def set_axon_ntff_profile_hook(h):
            hooks_mod._hook = h

        def get_axon_ntff_profile_hook():
            return hooks_mod._hook

        hooks_mod.set_axon_ntff_profile_hook = set_axon_ntff_profile_hook
        hooks_mod.get_axon_ntff_profile_hook = get_axon_ntff_profile_hook
        sys.modules["antenv.axon_hooks"] = hooks_mod
        import antenv as _a

        _a.axon_hooks = hooks_mod
        from trn_agent_boot.trn_boot import _ntff_profile_via_ctypes

        set_axon_ntff_profile_hook(
            _ntff_profile_via_ctypes("/opt/axon/libaxon_pjrt.so")
        )
    except Exception:
        pass


_install_ntff_hook()


def _install_verbose_cc_hook():
    """Wrap the PJRT->python compile callback so real tracebacks surface
    instead of an opaque 'CallFunctionObjArgs' error."""
    try:
        import traceback

        from concourse import bass2jax

        bass2jax.install_neuronx_cc_hook()
        import libneuronxla

        if getattr(libneuronxla, "_ant_verbose_wrap", False):
            return
        orig = libneuronxla.neuronx_cc

        def wrapped(*a, **k):
            try:
                return orig(*a, **k)
            except BaseException:
                traceback.print_exc()
                sys.stderr.flush()
                raise

        libneuronxla.neuronx_cc = wrapped
        libneuronxla._ant_verbose_wrap = True
        bass2jax.install_neuronx_cc_hook = lambda: None
    except Exception:
        pass


import bass_rust
import ml_dtypes
import concourse.bass as bass
import concourse.tile as tile
from concourse import mybir
from concourse.bass_utils import run_bass_kernel_spmd
from concourse.vector_clock import ScopedClock

BF16 = ml_dtypes.bfloat16

N_CORES = 8
B = 16  # batches total
S = 256  # seq len
D = 3800  # feature dim
H = 512  # hidden
C = 10  # classes

BLOC = B // N_CORES  # batches per core = 2
T = BLOC * S  # tokens per core = 512
DP = 3840  # D padded (+1 bias feature, up to 30*128)
KC = DP // 128  # 30 contraction chunks
ET = DP // 128  # 30 e-tiles of 128
PAIRS = KC // 2  # 15 DoubleRow chunk pairs
F32 = mybir.dt.float32
BF = mybir.dt.bfloat16
F8 = mybir.dt.float8e4
F8NP = mybir.dt.np(F8)  # ml_dtypes.float8_e4m3
# fp8 scale factors: weights are ~U(+-1/sqrt(3800)) which lands in e4m3's
# subnormal range, so weights are scaled up and the product scales are
# folded back out downstream (softmax scale / W1 scale).
WSCALE = 64.0  # on Wk, Wq, Wv
XASCALE = 16.0  # on abar (via the ones vector), so xa fits e4m3 nicely
SC_SCALE = WSCALE * WSCALE  # scores' = 4096 * scores
H_SCALE = WSCALE * XASCALE  # hT' = 1024 * h


class SplitDrainTileContext(tile.TileContext):
    """This walrus build rejects >1 sync-wait on the tail Drain; split the
    global-clock waits across a chain of single-wait drain instructions."""

    MAXW = 1

    def _drain_and_barrier(self, tick_clock, wait_clock):
        nc = self.nc
        drain_inst = nc.sync.drain()
        wait_clock.add_sem_waits(
            drain_inst.ins, ScopedClock({None: tick_clock.global_clock})
        )
        si = drain_inst.ins.sync_info
        if si is not None and si.on_wait and len(si.on_wait) > self.MAXW:
            waits = list(si.on_wait)
            si.on_wait = waits[: self.MAXW]
            rest = waits[self.MAXW :]
            for i in range(0, len(rest), self.MAXW):
                extra = nc.sync.drain()
                extra.ins.sync_info = bass_rust.SyncInfo(
                    on_wait=rest[i : i + self.MAXW], on_update=[]
                )
        nc.all_engine_barrier()
        assert self.sems is not None
        popped = nc._tile_sem_poison_stack.pop()
        assert popped is self._sem_poison
        nc.clear_and_free_semaphores(list(self.sems.allocated().values()))
        nc.all_engine_barrier()


def _fix_excess_waits(nc, aux_sem, maxw=1):
    """Walrus in this image rejects instructions with more than ~1 sync
    wait. Compute-engine instructions: hoist extra waits onto same-engine
    no-ops inserted just before (sequencers execute in order). DMACopy:
    its waits live in the DGE queue descriptor, so an SP-side chain waits
    on all the original conditions, bumps `aux_sem`, and the descriptor
    waits on aux_sem alone."""
    aux_count = 0
    for f in nc.m.functions:
        for bb in f.blocks:
            insts = bb.instructions
            if not any(
                i.sync_info and i.sync_info.on_wait
                and len(i.sync_info.on_wait) > maxw
                for i in insts
            ):
                continue
            out = []
            for ins in insts:
                si = ins.sync_info
                nw = len(si.on_wait) if si and si.on_wait else 0
                if nw > maxw:
                    waits = list(si.on_wait)
                    if isinstance(ins, mybir.InstDMACopy):
                        for j, w in enumerate(waits):
                            nop = mybir.InstNoOp(name=f"{ins.name}-dw{j}")
                            nop.engine = mybir.EngineType.SP
                            nop.sync_info = bass_rust.SyncInfo(
                                on_wait=[w], on_update=[]
                            )
                            out.append(nop)
                        aux_count += 1
                        inc = mybir.InstNoOp(name=f"{ins.name}-dinc")
                        inc.engine = mybir.EngineType.SP
                        inc.sync_info = bass_rust.SyncInfo(
                            on_wait=[],
                            on_update=[
                                bass_rust.SyncUpdate(
                                    sync_type="semaphore",
                                    id=aux_sem.num,
                                    ant_name=aux_sem.name,
                                    update_mode="sem-add-imm",
                                    update_value=1,
                                    update_reg=None,
                                )
                            ],
                        )
                        out.append(inc)
                        si.on_wait = [
                            bass_rust.SyncWait(
                                sync_type="semaphore",
                                id=aux_sem.num,
                                ant_name=aux_sem.name,
                                wait_mode="sem-ge-imm",
                                wait_value=aux_count,
                                wait_reg=None,
                            )
                        ]
                    else:
                        keep = waits[-maxw:]
                        rest = waits[:-maxw]
                        for j, w in enumerate(rest):
                            nop = mybir.InstNoOp(name=f"{ins.name}-xw{j}")
                            nop.engine = ins.engine
                            nop.sync_info = bass_rust.SyncInfo(
                                on_wait=[w], on_update=[]
                            )
                            out.append(nop)
                        si.on_wait = keep
                out.append(ins)
            bb.instructions = out
    if aux_count:
        # reset aux sem at the very end so a re-executed NEFF starts clean
        f = nc.m.functions[0]
        bb = list(f.blocks)[-1]
        rst = mybir.InstNoOp(name="auxwait-reset")
        rst.engine = mybir.EngineType.SP
        rst.sync_info = bass_rust.SyncInfo(
            on_wait=[],
            on_update=[
                bass_rust.SyncUpdate(
                    sync_type="semaphore",
                    id=aux_sem.num,
                    ant_name=aux_sem.name,
                    update_mode="sem-sub-imm",
                    update_value=aux_count,
                    update_reg=None,
                )
            ],
        )
        il = bb.instructions
        il.append(rst)
        bb.instructions = il


def build_kernel() -> bass.Bass:
    nc = bass.Bass()

    x_d = nc.declare_dram_parameter("x8", [128, PAIRS, 2, T], F8, isOutput=False)
    xtok_d = nc.declare_dram_parameter("xtok", [128, 4, DP], BF, isOutput=False)
    wk_d = nc.declare_dram_parameter("wk", [ET, 128, PAIRS, 2, 128], F8,
                                     isOutput=False)
    wq_d = nc.declare_dram_parameter("wq", [ET, 128, PAIRS, 2, 128], F8,
                                     isOutput=False)
    wv_d = nc.declare_dram_parameter("wv", [ET, 128, KC, 128], F8, isOutput=False)
    w1_d = nc.declare_dram_parameter("w1", [128, KC, H], BF, isOutput=False)
    w2_d = nc.declare_dram_parameter("w2", [128, 5, H], F32, isOutput=False)
    w3_d = nc.declare_dram_parameter("w3", [128, 5, C], F32, isOutput=False)
    e0b_d = nc.declare_dram_parameter("e0b", [128, BLOC], F32, isOutput=False)
    out_d = nc.declare_dram_parameter("outT", [C, BLOC], F32, isOutput=True)

    aux_sem = nc.alloc_semaphore("auxwait")
    with SplitDrainTileContext(nc) as tc:
        with tc.tile_pool(name="persist", bufs=1) as persist:
            _emit(nc, tc, persist, x_d, xtok_d, wk_d, wq_d, wv_d, w1_d, w2_d,
                  w3_d, e0b_d, out_d)
    _fix_excess_waits(nc, aux_sem)
    return nc


def _emit(nc, tc, persist, x_d, xtok_d, wk_d, wq_d, wv_d, w1_d, w2_d, w3_d,
          e0b_d, out_d):
    # ------------------ persistent tiles ------------------
    # x8 split per DoubleRow pair so early matmuls only wait on their own
    # slice's DMA (Tile dependencies are whole-tile).
    x8c = [persist.tile([128, 2, T], F8, name=f"x8{p}", tag=f"x8{p}")
           for p in range(PAIRS)]
    ones_s = persist.tile([128, 1], BF)
    nc.vector.memset(ones_s[:], XASCALE / S)
    a_bar2 = persist.tile([128, 4, BLOC], BF)
    nc.vector.memset(a_bar2[:], 0.0)
    x_tok = persist.tile([128, 4, DP], BF)
    xaT = persist.tile([128, KC, BLOC], F8)
    hT = persist.tile([128, KC, BLOC], BF)

    # MLP weights: tiles up-front, DMAs issued a few iterations into
    # phase 1 so they overlap compute instead of the critical startup
    mlpw = tc.alloc_tile_pool(name="mlpw", bufs=1)
    w1_t = mlpw.tile([128, KC, H], BF)
    w2_t = mlpw.tile([128, 5, H], F32)
    w3_t = mlpw.tile([128, 5, C], F32)

    # ------------- phase 1: k/q projections + score accumulation -------------
    DR = mybir.MatmulPerfMode.DoubleRow
    with tc.tile_pool(name="psum_sc", bufs=1, space="PSUM") as psum_sc:
        ps = [
            psum_sc.tile([128, S], F32, name=f"scores{i}", tag=f"scores{i}")
            for i in range(4)  # index = 2*b + it
        ]
        with (
            tc.tile_pool(name="wkq", bufs=1) as wkq_pool,
            tc.tile_pool(name="kq_sb", bufs=1) as kq_sb,
            tc.tile_pool(name="psum_kq", bufs=1, space="PSUM") as psum_kq,
        ):
            for et in range(ET):
                wk_t = wkq_pool.tile([128, PAIRS, 2, 128], F8, tag="wk", bufs=2)
                nc.sync.dma_start(wk_t[:], wk_d[et])
                wq_t = wkq_pool.tile([128, PAIRS, 2, 128], F8, tag="wq", bufs=2)
                nc.sync.dma_start(wq_t[:], wq_d[et])
                if et == 0:
                    # x8 DMAs after wk0/wq0 so the first weight block isn't
                    # queued behind them
                    for p in range(PAIRS):
                        nc.sync.dma_start(x8c[p][:], x_d[:, p])
                if et == 3:
                    for tt in range(4):
                        nc.sync.dma_start(x_tok[:, tt, :], xtok_d[:, tt, :])
                    nc.sync.dma_start(w2_t[:], w2_d[:])
                    nc.sync.dma_start(w3_t[:], w3_d[:])
                if 6 <= et < 6 + KC // 2:
                    # spread the 30 W1 chunk loads over phase-1 iterations
                    kc0 = 2 * (et - 6)
                    nc.sync.dma_start(w1_t[:, kc0 : kc0 + 2, :],
                                      w1_d[:, kc0 : kc0 + 2, :])

                pk = psum_kq.tile([128, T], F32, tag="pk", bufs=2)
                for p in range(PAIRS):
                    nc.tensor.matmul(
                        pk[:], wk_t[:, p], x8c[p][:],
                        start=(p == 0), stop=(p == PAIRS - 1),
                        perf_mode=DR,
                    )
                k_et = kq_sb.tile([128, T], BF, tag="k_et", bufs=2)
                nc.vector.tensor_copy(k_et[:], pk[:])

                pq = psum_kq.tile([128, T], F32, tag="pq", bufs=2)
                for p in range(PAIRS):
                    nc.tensor.matmul(
                        pq[:], wq_t[:, p], x8c[p][:],
                        start=(p == 0), stop=(p == PAIRS - 1),
                        perf_mode=DR,
                    )
                q_et = kq_sb.tile([128, T], BF, tag="q_et", bufs=2)
                nc.vector.tensor_copy(q_et[:], pq[:])

                for b in range(BLOC):
                    for it in range(2):
                        nc.tensor.matmul(
                            ps[2 * b + it][:],
                            q_et[:, b * S + it * 128 : b * S + (it + 1) * 128],
                            k_et[:, b * S : (b + 1) * S],
                            start=(et == 0), stop=(et == ET - 1),
                            skip_group_check=True,
                        )

        # ------------- phase 2: softmax + abar (column means) -------------
        with (
            tc.tile_pool(name="smx", bufs=1) as smx,
            tc.tile_pool(name="psum_ab", bufs=1, space="PSUM") as psum_ab,
        ):
            pab = [
                psum_ab.tile([128, 1], F32, name=f"pab{i}", tag=f"pab{i}")
                for i in range(4)  # index = 2*b + jc
            ]
            for b in range(BLOC):
                for it in range(2):
                    p = ps[2 * b + it]
                    mx = smx.tile([128, 1], F32, tag="mx", bufs=2)
                    nc.vector.reduce_max(
                        out=mx[:], in_=p[:], axis=mybir.AxisListType.X
                    )
                    negm = smx.tile([128, 1], F32, tag="negm", bufs=2)
                    nc.vector.tensor_scalar_mul(negm[:], mx[:], -1.0 / SC_SCALE)
                    pexp = smx.tile([128, S], F32, tag="pexp", bufs=2)
                    sm = smx.tile([128, 1], F32, tag="sm", bufs=2)
                    nc.scalar.activation(
                        pexp[:], p[:], mybir.ActivationFunctionType.Exp,
                        bias=negm[:], scale=1.0 / SC_SCALE, accum_out=sm[:],
                    )
                    rin = smx.tile([128, 1], F32, tag="rin", bufs=2)
                    nc.vector.reciprocal(rin[:], sm[:])
                    attn_b = smx.tile([128, S], BF, tag="attn", bufs=2)
                    nc.vector.tensor_scalar_mul(attn_b[:], pexp[:], rin[:])
                    for jc in range(2):
                        nc.tensor.matmul(
                            pab[2 * b + jc][:],
                            attn_b[:, jc * 128 : (jc + 1) * 128],
                            ones_s[:],
                            start=(it == 0), stop=(it == 1),
                            skip_group_check=True,
                        )
            for b in range(BLOC):
                for jc in range(2):
                    nc.vector.tensor_copy(
                        a_bar2[:, 2 * b + jc, b : b + 1], pab[2 * b + jc][:]
                    )

    # ------------------ phase 3: xa = abar @ x ------------------
    with tc.tile_pool(name="psum_xa", bufs=1, space="PSUM") as psum_xa:
        for dt in range(KC):
            pxa = psum_xa.tile([128, BLOC], F32, tag="pxa", bufs=2)
            for tt in range(4):
                nc.tensor.matmul(
                    pxa[:],
                    x_tok[:, tt, dt * 128 : (dt + 1) * 128],
                    a_bar2[:, tt, :],
                    start=(tt == 0), stop=(tt == 3),
                )
            nc.vector.tensor_copy(xaT[:, dt, :], pxa[:])

    # ------------------ phase 4: hT = Wv^T-tiles @ xaT ------------------
    with (
        tc.tile_pool(name="wv", bufs=1) as wv_pool,
        tc.tile_pool(name="psum_h", bufs=1, space="PSUM") as psum_h,
    ):
        for et in range(ET):
            wv_t = wv_pool.tile([128, KC, 128], F8, tag="wv", bufs=8)
            nc.sync.dma_start(wv_t[:], wv_d[et])
            ph = psum_h.tile([128, BLOC], F32, tag="ph", bufs=2)
            for kc in range(KC):
                nc.tensor.matmul(
                    ph[:], wv_t[:, kc, :], xaT[:, kc, :],
                    start=(kc == 0), stop=(kc == KC - 1),
                )
            nc.vector.tensor_copy(hT[:, et, :], ph[:])

    # ------------------ phase 5: MLP (fp32) ------------------
    with (
        tc.tile_pool(name="mlph", bufs=1) as mlph,
        tc.tile_pool(name="psum_m", bufs=1, space="PSUM") as psum_m,
    ):
        h1T = mlph.tile([128, 5, BLOC], F32)
        nc.sync.dma_start(h1T[:, 4, :], e0b_d[:])
        for ot in range(4):
            pm = psum_m.tile([128, BLOC], F32, tag="pm1", bufs=2)
            for kc in range(KC):
                nc.tensor.matmul(
                    pm[:],
                    w1_t[:, kc, ot * 128 : (ot + 1) * 128],
                    hT[:, kc, :],
                    start=(kc == 0), stop=(kc == KC - 1),
                )
            nc.scalar.activation(
                h1T[:, ot, :], pm[:], mybir.ActivationFunctionType.Sigmoid
            )

        h2T = mlph.tile([128, 5, BLOC], F32)
        nc.sync.dma_start(h2T[:, 4, :], e0b_d[:])
        for ot in range(4):
            pm = psum_m.tile([128, BLOC], F32, tag="pm2", bufs=2)
            for oc in range(5):
                nc.tensor.matmul(
                    pm[:],
                    w2_t[:, oc, ot * 128 : (ot + 1) * 128],
                    h1T[:, oc, :],
                    start=(oc == 0), stop=(oc == 4),
                )
            nc.scalar.activation(
                h2T[:, ot, :], pm[:], mybir.ActivationFunctionType.Sigmoid
            )

        pm3 = psum_m.tile([C, BLOC], F32, tag="pm3")
        for oc in range(5):
            nc.tensor.matmul(
                pm3[:],
                w3_t[:, oc, :],
                h2T[:, oc, :],
                start=(oc == 0), stop=(oc == 4),
            )
        out_sb = mlph.tile([C, BLOC], F32)
        nc.vector.tensor_copy(out_sb[:], pm3[:])
        nc.sync.dma_start(out_d[:], out_sb[:])
    mlpw.release()


# ---------------------------------------------------------------------------
# Host-side packing
# ---------------------------------------------------------------------------
def _pack_qkv8(W, bias, scale=1.0):
    """W [D, D] (rows e out, cols d in), bias [D] -> DoubleRow-interleaved
    [ET, 128, PAIRS, 2, 128] e4m3 with
    A[et, dp, p, ko, ep] = WSCALE * Wp[et*128+ep, (2p+ko)*128+dp];
    bias in column d=3800."""
    Wp = np.zeros((DP, DP), dtype=np.float32)
    Wp[:D, :D] = W * (WSCALE * scale)
    Wp[:D, D] = bias * (WSCALE * scale)
    A = Wp.reshape(ET, 128, PAIRS, 2, 128).transpose(0, 4, 2, 3, 1)
    return np.ascontiguousarray(A, dtype=F8NP)


def _pack_wv8(W, bias):
    """W [D, D], bias [D] -> [ET, 128, KC, 128] e4m3 with
    A[et, dp, kc, ep] = WSCALE * Wp[et*128+ep, kc*128+dp]; bias in column
    d=3800; unit row at e=3800 propagates the bias feature into hT."""
    Wp = np.zeros((DP, DP), dtype=np.float32)
    Wp[:D, :D] = W * WSCALE
    Wp[:D, D] = bias * WSCALE
    Wp[D, D] = WSCALE
    A = Wp.reshape(ET, 128, KC, 128).transpose(0, 3, 2, 1)
    return np.ascontiguousarray(A, dtype=F8NP)


def _pack_x8(xc):
    """xc [BLOC, S, D] -> [128, PAIRS, 2, T] e4m3, bias row d=3800 = 1."""
    xt = np.zeros((DP, T), dtype=np.float32)
    xt[:D, :] = xc.reshape(T, D).T
    xt[D, :] = 1.0
    A = xt.reshape(PAIRS, 2, 128, T).transpose(2, 0, 1, 3)
    return np.ascontiguousarray(A, dtype=F8NP)


def _pack_xtok(xc):
    """xc [BLOC, S, D] -> [128, 4, DP] bf16 (token partition), col d=3800 = 1."""
    xp = np.zeros((T, DP), dtype=np.float32)
    xp[:, :D] = xc.reshape(T, D)
    xp[:, D] = 1.0
    A = xp.reshape(4, 128, DP).transpose(1, 0, 2)
    return np.ascontiguousarray(A, dtype=BF16)


def _pack_w1(W1, b1):
    """W1 [H, D] -> [128, KC, H] bf16: A[dp, kc, o] = W1p[o, kc*128+dp] with
    the hT scale (1/H_SCALE) folded in; b1 in column d=3800 (hT[3800] ==
    H_SCALE)."""
    W1p = np.zeros((H, DP), dtype=np.float32)
    W1p[:, :D] = W1 / np.float32(H_SCALE)
    W1p[:, D] = b1 / np.float32(H_SCALE)
    A = W1p.T.reshape(KC, 128, H).transpose(1, 0, 2)
    return np.ascontiguousarray(A, dtype=BF16)


def _pack_w2(W2, b2):
    A = np.zeros((128, 5, H), dtype=np.float32)
    A[:, :4, :] = W2.T.reshape(4, 128, H).transpose(1, 0, 2)
    A[0, 4, :] = b2
    return np.ascontiguousarray(A)


def _pack_w3(W3, b3):
    A = np.zeros((128, 5, C), dtype=np.float32)
    A[:, :4, :] = W3.T.reshape(4, 128, C).transpose(1, 0, 2)
    A[0, 4, :] = b3
    return np.ascontiguousarray(A)


_NC_CACHE = {}


def _get_nc():
    if "nc" not in _NC_CACHE:
        _NC_CACHE["nc"] = build_kernel()
    return _NC_CACHE["nc"]


def kernel(x, Wk, bk, Wq, bq, Wv, bv, W1, b1, W2, b2, W3, b3, _trace=False):
    x = np.asarray(x, dtype=np.float32)
    scale = float(1.0 / np.sqrt(np.float32(D)))

    wk_p = _pack_qkv8(np.asarray(Wk, np.float32), np.asarray(bk, np.float32))
    wq_p = _pack_qkv8(
        np.asarray(Wq, np.float32), np.asarray(bq, np.float32), scale=scale
    )
    wv_p = _pack_wv8(np.asarray(Wv, np.float32), np.asarray(bv, np.float32))
    w1_p = _pack_w1(np.asarray(W1, np.float32), np.asarray(b1, np.float32))
    w2_p = _pack_w2(np.asarray(W2, np.float32), np.asarray(b2, np.float32))
    w3_p = _pack_w3(np.asarray(W3, np.float32), np.asarray(b3, np.float32))
    e0b = np.zeros((128, BLOC), dtype=np.float32)
    e0b[0, :] = 1.0

    in_maps = []
    for c in range(N_CORES):
        xc = x[c * BLOC : (c + 1) * BLOC]
        in_maps.append(
            {
                "x8": _pack_x8(xc),
                "xtok": _pack_xtok(xc),
                "wk": wk_p,
                "wq": wq_p,
                "wv": wv_p,
                "w1": w1_p,
                "w2": w2_p,
                "w3": w3_p,
                "e0b": e0b,
            }
        )

    nc = _get_nc()
    _install_verbose_cc_hook()
    res = run_bass_kernel_spmd(nc, in_maps, list(range(N_CORES)), trace=_trace)
    out = np.zeros((B, C), dtype=np.float32)
    for c in range(N_CORES):
        out[c * BLOC : (c + 1) * BLOC] = res.results[c]["outT"].T
    if _trace:
        return out, res
    return out
